# revision 2
# baseline (speedup 1.0000x reference)
"""Stack-style neural memory kernel for Trainium2 (8 NeuronCores, SPMD).

Reference semantics: at step t, push (d1,v1),(d2,v2); read up to total
strength u_t from the top of the stack; pop strength u_t.  The read
summary is linear in the pushed values:

    out[t,b,:] = sum_j W[t,j,b] * V[j,b,:]      (j = slot index, 2T slots)

where the weights W depend only on the (T,B,1)-sized strength tensors
(u,d1,d2).  W is computed on host (tiny sequential bookkeeping, ~4M
scalar ops; it also needs a global max over the whole batch, which would
otherwise force cross-core communication).  The device does the
memory-heavy part: per batch element a (T x 2T) @ (2T x R) matmul,
batch-parallel across 8 cores with no communication.

Everything on device is bf16 (inputs W,V and the output; PSUM
accumulates in fp32) -- the rel-err budget is 2e-2 and bf16 costs
~0.2%.  W[t,j] = 0 for j > 2t+1, so the second k-chunk of W (slots
128..255) is packed only for t >= 64; output rows 0:64 take one matmul,
rows 64:128 take two.  Per-core HBM traffic: V 4MB + W 0.75MB in,
out 2MB  ->  memory-bound, ~19us roofline at 358 GB/s.
"""

import ml_dtypes
import numpy as np

BF16 = ml_dtypes.bfloat16

T, B, R = 128, 128, 512
NSLOTS = 2 * T
N_CORES = 8
BSH = B // N_CORES  # batch shard per core
GRP = 2             # batches per DMA group
NGRP = BSH // GRP
M2 = 64             # chunk1 lhsT columns kept (t in [64,128): W zero for t<64)
WM = 128 + M2       # per-batch lhsT columns: chunk0 (all t) + chunk1 (t>=64)
BW = WM + 2 * R     # per-batch fused row: lhsT pack + both v chunks

_NC_CACHE = {}


def _compute_weights(u, d1, d2):
    """W[t, j, b]: read weight of slot j at step t (float32 (T, 2T, B))."""
    uu = u[:, :, 0]
    S = np.zeros((NSLOTS, B), np.float32)
    W = np.empty((T, NSLOTS, B), np.float32)
    for t in range(T):
        S[2 * t] = d1[t, :, 0]
        S[2 * t + 1] = d2[t, :, 0]
        # strength of slots above j (stack top = highest index first)
        c = np.cumsum(S[::-1], axis=0)[::-1]
        cum = c - S
        avail = uu[t][None, :] - cum
        # reference takes a GLOBAL max over the batch for the read scale
        scal = avail.max(axis=1)
        Wt = np.minimum(S, scal[:, None])
        Wt[2 * t + 2:] = 0.0  # slots not yet pushed hold V=0 in the reference
        W[t] = Wt
        # pop u_t: elementwise depletion, same slot order, same cum
        S -= np.minimum(S, np.maximum(0.0, avail))
    return W


def _build_nc(reps=1, loop_n=1):
    import contextlib

    from concourse import bacc, tile, mybir

    DT = mybir.dt.bfloat16
    PS = mybir.dt.float32
    nc = bacc.Bacc(None)
    # One fused, fully partition-contiguous load stream per group:
    # wv[g, k, bi*BW + m]: m<128 chunk0 lhsT (t=m), 128<=m<192 chunk1
    # lhsT (t=64+m-128), 192<=m<704 v chunk0 row, 704<=m<1216 v chunk1
    # row.  Per-partition contiguous run = GRP*BW*2 bytes.
    wv = nc.declare_dram_parameter("wv", [NGRP, 128, GRP * BW], DT, isOutput=False)
    # output t-major per group: o[g, t, bi*512 + r]
    o = nc.declare_dram_parameter("o", [NGRP, 128, GRP * 512], DT, isOutput=True)

    with tile.TileContext(nc) as tc:
        with (
            tc.tile_pool(name="wvp", bufs=4) as wvp,
            tc.tile_pool(name="op", bufs=4) as op,
            tc.tile_pool(name="ps", bufs=8, space="PSUM") as ps,
        ):
            loop_cm = (
                tc.For_i(0, loop_n, 1) if loop_n > 1 else contextlib.nullcontext()
            )
            with loop_cm:
                for rep in range(reps):
                    for g in range(NGRP):
                        # alternate the two HWDGE rings (SP / Act) by group
                        # parity so loads and stores stream on both rings
                        ld = nc.sync if g % 2 == 0 else nc.scalar
                        ld2 = nc.scalar if g % 2 == 0 else nc.sync
                        st = nc.scalar if g % 2 == 0 else nc.sync
                        wv_t = wvp.tile([128, GRP, BW], DT, tag="wv")
                        # split each group load across BOTH HWDGE rings:
                        # halves the wait before the group's first matmul
                        wv_g = wv[g].rearrange("k (b m) -> k b m", m=BW)
                        ld.dma_start(wv_t[:, 0], wv_g[:, 0])
                        ld2.dma_start(wv_t[:, 1], wv_g[:, 1])
                        out_t = op.tile([128, GRP, 512], DT, tag="out")
                        for bi in range(GRP):
                            vc0 = wv_t[:, bi, WM:WM + 512]
                            vc1 = wv_t[:, bi, WM + 512:WM + 1024]
                            acc = ps.tile([128, 512], PS)
                            # rows t<64: only slots j<128 are live
                            nc.tensor.matmul(
                                acc[0:64],
                                wv_t[:, bi, 0:64],
                                vc0,
                                start=True,
                                stop=True,
                            )
                            # rows t>=64: both slot chunks
                            nc.tensor.matmul(
                                acc[64:128],
                                wv_t[:, bi, 64:128],
                                vc0,
                                start=True,
                                stop=False,
                            )
                            nc.tensor.matmul(
                                acc[64:128],
                                wv_t[:, bi, 128:128 + M2],
                                vc1,
                                start=False,
                                stop=True,
                            )
                            nc.vector.tensor_copy(out_t[:, bi], acc[:])
                        st.dma_start(o[g], out_t[:])
    nc.compile()
    return nc


def _make_in_maps(u, d1, d2, v1, v2):
    W = _compute_weights(u, d1, d2)  # (T, 2T, B)

    Vfull = np.empty((NSLOTS, B, R), np.float32)
    Vfull[0::2] = v1
    Vfull[1::2] = v2

    in_maps = []
    for c in range(N_CORES):
        gb = slice(c * BSH, (c + 1) * BSH)
        # fused per-batch row [k, m]: 0:128 = W[m, k, b] (chunk0 lhsT),
        # 128:192 = W[64+(m-128), 128+k, b] (chunk1 lhsT, t>=64 only),
        # 192:704 = Vfull[k, b, r], 704:1216 = Vfull[128+k, b, r].
        Wc = W[:, :, gb]          # (T, 256, BSH)
        Vc = Vfull[:, gb, :]      # (256, BSH, R)
        pack = np.empty((BSH, 128, BW), np.float32)
        pack[:, :, 0:128] = Wc[:, 0:128, :].transpose(2, 1, 0)
        pack[:, :, 128:WM] = Wc[64:128, 128:256, :].transpose(2, 1, 0)
        pack[:, :, WM:WM + 512] = Vc[0:128].transpose(1, 0, 2)
        pack[:, :, WM + 512:] = Vc[128:256].transpose(1, 0, 2)
        wvc = np.ascontiguousarray(
            pack.reshape(NGRP, GRP, 128, BW).transpose(0, 2, 1, 3)
        ).reshape(NGRP, 128, GRP * BW).astype(BF16)
        in_maps.append({"wv": wvc})
    return in_maps


def kernel(u, d1, d2, v1, v2):
    from concourse.bass_utils import run_bass_kernel_spmd

    u = np.ascontiguousarray(np.asarray(u, np.float32))
    d1 = np.ascontiguousarray(np.asarray(d1, np.float32))
    d2 = np.ascontiguousarray(np.asarray(d2, np.float32))
    v1 = np.ascontiguousarray(np.asarray(v1, np.float32))
    v2 = np.ascontiguousarray(np.asarray(v2, np.float32))

    in_maps = _make_in_maps(u, d1, d2, v1, v2)

    if "nc" not in _NC_CACHE:
        _NC_CACHE["nc"] = _build_nc()
    res = run_bass_kernel_spmd(_NC_CACHE["nc"], in_maps, list(range(N_CORES)))

    # o[g, t, bi*512 + r] per core  ->  out[t, b_global, r]
    out = np.concatenate(
        [
            res.results[c]["o"]
            .astype(np.float32)
            .reshape(NGRP, T, GRP, R)
            .transpose(1, 0, 2, 3)
            .reshape(T, BSH, R)
            for c in range(N_CORES)
        ],
        axis=1,
    )
    return np.ascontiguousarray(out)


if __name__ == "__main__":
    rng = np.random.default_rng(0)
    ins = {
        "u": rng.random((T, B, 1), dtype=np.float32),
        "d1": rng.random((T, B, 1), dtype=np.float32),
        "d2": rng.random((T, B, 1), dtype=np.float32),
        "v1": rng.standard_normal((T, B, R), dtype=np.float32),
        "v2": rng.standard_normal((T, B, R), dtype=np.float32),
    }
    out = kernel(**ins)
    print(out.shape, out.dtype)
